# revision 10
# baseline (speedup 1.0000x reference)
"""Trainium2 Bass kernel for ClassicAttention (B=2, S=2048, D=1024, H=16).

Sharding: tensor-parallel over heads across 8 cores (2 heads/core).
  - Host pre-transposes x to x^T [D, M] and pre-casts all matmul operands
    to bf16; biases arrive as pre-broadcast SBUF tiles so no K=1 seed
    matmuls are needed (bias fused into the PSUM evacuation add).
  - QKV projection: each core computes Q^T,K^T (d-major) and V (row-major)
    for its 2 heads over all B*S rows straight from x^T in SBUF.
  - Attention: transposed-scores formulation S^T[k,q]; both heads share one
    [128,1024] score tile so each k-tile needs a single exp ACTIVATE.  The
    softmax denominator rides row 64 of the AV accumulator via a ones
    column in V.  AV matmuls are causally trimmed to [qo:512] like the
    scores (skip_group_check since the trimmed stop flag only covers the
    tail region).  Per-kt software pipeline: scores(kt+1) is emitted before
    AV(kt) so the exp(kt) runs while the PE does useful work.  Per-q-group
    normalization broadcasts the sums row with a K=1 ones matmul.
  - c_proj: 3 merged AllGathers of bf16 context (b0 whole: 512KB/rank;
    b1 in two halves: 256KB/rank) instead of 8 small ones -- the collective
    engine serializes AGs at a high fixed cost, so fewer/bigger ops keep it
    off the critical path.  Gathers land on the sync DMA queue; each core
    computes a 128-column slice of the output, transposed ([j, B*S]); the
    host transposes back.
  - Emission order software-pipelines phases: QKV(b1) matmuls interleave
    into attention(b0)'s PE stream, c_proj units into attention(b1).
  - Input DMAs spread across engine queues so QKV(b0) starts ~5us in.
All matmuls bf16 inputs with fp32 PSUM accumulation.
"""

import numpy as np
import ml_dtypes

import concourse.bass as bass
import concourse.tile as tile
import concourse.mybir as mybir
from concourse import bacc
from concourse.bass_utils import run_bass_kernel_spmd

F32 = mybir.dt.float32
BF16 = mybir.dt.bfloat16

NCORES = 8
B, S, D = 2, 2048, 1024
H, HD = 16, 64
HPC = H // NCORES          # heads per core = 2
M = B * S                  # 4096 rows
ST_B = S // 128            # 16 s-tiles per batch
KCH = D // 128             # 8 contraction chunks
G_PER_B = S // 512         # 4 q-supers per batch
SCALE = 1.0 / (HD ** 0.5)
EXP = mybir.ActivationFunctionType.Exp


def build_ir(nc):
    # ---------------- DRAM I/O ----------------
    xt = nc.dram_tensor("xt", [D, M], BF16, kind="ExternalInput").ap()
    wqk = nc.dram_tensor("wqk", [D, 256], BF16, kind="ExternalInput").ap()
    wv = nc.dram_tensor("wv", [D, 128], BF16, kind="ExternalInput").ap()
    wp = nc.dram_tensor("wp", [D, 128], BF16, kind="ExternalInput").ap()
    bqk = nc.dram_tensor("bqk", [256], F32, kind="ExternalInput").ap()
    bqk_bc = nc.dram_tensor("bqk_bc", [128, 2, 512], F32,
                            kind="ExternalInput").ap()
    bv_bc = nc.dram_tensor("bv_bc", [128, 2, 64], BF16,
                           kind="ExternalInput").ap()
    bp_bc = nc.dram_tensor("bp_bc", [128, 512], F32,
                           kind="ExternalInput").ap()
    outT = nc.dram_tensor("outT", [128, M], F32, kind="ExternalOutput").ap()

    # causal mask for the diagonal 128-block: mask[k, c] = 1 if c >= k,
    # duplicated for both heads ([128, 2, 128]) so one mul covers a kt tile
    mask_np = (np.arange(128)[None, :] >= np.arange(128)[:, None])
    mask2 = np.stack([mask_np, mask_np], axis=1)
    mask_const = nc.inline_tensor(mask2.astype(ml_dtypes.bfloat16),
                                  "mask_const").ap()

    rg = [list(range(NCORES))]

    with tile.TileContext(nc) as tc:
        _emit(nc, tc, xt, wqk, wv, wp, bqk, bqk_bc, bv_bc, bp_bc, outT,
              mask_const, rg)
    return nc


def _emit(nc, tc, xt, wqk, wv, wp, bqk, bqk_bc, bv_bc, bp_bc, outT,
          mask_const, rg):
    import contextlib
    es = contextlib.ExitStack()
    with es:
        singles = es.enter_context(tc.tile_pool(name="singles", bufs=1))
        dram = es.enter_context(tc.tile_pool(name="dram", bufs=1, space="DRAM"))

        # ------------- persistent SBUF -------------
        qT = singles.tile([128, M], BF16, tag="qT")
        kT = singles.tile([128, M], BF16, tag="kT")
        v_sb = singles.tile([128, B * ST_B, 130], BF16, tag="v_sb")
        mask_sb = singles.tile([128, 2, 128], BF16, tag="mask_sb")
        wqk_sb = singles.tile([128, KCH, 256], BF16, tag="wqk_sb")
        wv_sb = singles.tile([128, KCH, 128], BF16, tag="wv_sb")
        wp_sb = singles.tile([128, KCH, 128], BF16, tag="wp_sb")
        bqk_sb = singles.tile([128, 2], F32, tag="bqk_sb")
        bqk_bc_sb = singles.tile([128, 2, 512], F32, tag="bqk_bc_sb")
        bv_bc_sb = singles.tile([128, 2, 64], BF16, tag="bv_bc_sb")
        bp_bc_sb = singles.tile([128, 512], F32, tag="bp_bc_sb")
        ones_p64 = singles.tile([65, 64], F32, tag="ones_p64")
        ones512 = singles.tile([1, 512], BF16, tag="ones512")

        # input DMAs spread across the three DMA-capable queues (sync,
        # scalar, gpsimd) so everything streams in parallel from t=0:
        # weights on scalar (ready ~2us), x^T b0 on sync, x^T b1 + mask on
        # gpsimd, small bias tiles behind the weights on scalar
        nc.scalar.dma_start(out=wqk_sb,
                            in_=wqk.rearrange("(c p) j -> p c j", p=128))
        nc.scalar.dma_start(out=wv_sb,
                            in_=wv.rearrange("(c p) j -> p c j", p=128))
        nc.scalar.dma_start(out=wp_sb,
                            in_=wp.rearrange("(c p) j -> p c j", p=128))
        nc.scalar.dma_start(out=bqk_sb,
                            in_=bqk.rearrange("(t p) -> p t", p=128))
        nc.scalar.dma_start(out=bqk_bc_sb, in_=bqk_bc)
        nc.scalar.dma_start(out=bv_bc_sb, in_=bv_bc)
        nc.scalar.dma_start(out=bp_bc_sb, in_=bp_bc)
        nc.gpsimd.dma_start(out=mask_sb, in_=mask_const)
        nc.vector.memset(ones512, 1.0)
        nc.vector.memset(ones_p64, 1.0)
        # ones columns of V (cols 64 and 129); data cols are written by evac
        nc.vector.memset(v_sb[:, :, 64:65], 1.0)
        nc.vector.memset(v_sb[:, :, 129:130], 1.0)

        # PE warmup: back-to-back K=1 matmuls keep the PE clock ramping
        # while the input DMAs stream
        with tc.tile_pool(name="warm_ps", bufs=1, space="PSUM") as warm_ps:
            wt = warm_ps.tile([128, 512], F32)
            for _ in range(10):
                nc.tensor.matmul(wt, lhsT=ones512[:, 0:128],
                                 rhs=ones512, start=True, stop=True)

        # x^T: two [128, 8, 2048] tiles (m-halves); b0 slices on the sync
        # queue, b1 slices on the gpsimd queue
        xt_r = xt.rearrange("(c p) m -> p c m", p=128)
        xt_h = {h: singles.tile([128, KCH, S], BF16, tag=f"xt_h{h}",
                                name=f"xt_h{h}") for h in range(2)}
        for su in range(8):
            h, o = su // 4, (su % 4) * 512
            eng = nc.sync if h == 0 else nc.gpsimd
            eng.dma_start(out=xt_h[h][:, :, o:o + 512],
                          in_=xt_r[:, :, h * S + o:h * S + o + 512])

        def xt_cols(c, m0, m1):
            """slice of x^T chunk c for global columns [m0, m1)"""
            h = m0 // S
            assert m1 <= (h + 1) * S
            return xt_h[h][:, c, m0 - h * S:m1 - h * S]

        # ------------- shared psum pools (8 banks total) -------------
        s_ps = es.enter_context(tc.tile_pool(name="s_ps", bufs=2, space="PSUM"))
        ctx_ps = es.enter_context(tc.tile_pool(name="ctx_ps", bufs=2, space="PSUM"))
        mm_ps = es.enter_context(tc.tile_pool(name="mm_ps", bufs=2, space="PSUM"))

        pt_pool = es.enter_context(tc.tile_pool(name="pt", bufs=4))
        row_pool = es.enter_context(tc.tile_pool(name="row", bufs=2))
        rec_pool = es.enter_context(tc.tile_pool(name="rec", bufs=2))
        cs_pool = es.enter_context(tc.tile_pool(name="cs", bufs=4))
        cg_pool = es.enter_context(tc.tile_pool(name="cg", bufs=3))
        osb = es.enter_context(tc.tile_pool(name="osb", bufs=3))

        # ------------- QKV emitters -------------
        def emit_qk(su, jt, use_act=False):
            """Q^T (jt=0) or K^T (jt=1) for row-super su (512 cols)."""
            dst = qT if jt == 0 else kT
            ps = mm_ps.tile([128, 512], F32, tag="mm")
            for kc in range(KCH):
                nc.tensor.matmul(
                    ps,
                    lhsT=wqk_sb[:, kc, jt * 128:(jt + 1) * 128],
                    rhs=xt_cols(kc, su * 512, (su + 1) * 512),
                    start=(kc == 0), stop=(kc == KCH - 1),
                )
            dslice = dst[:, su * 512:(su + 1) * 512]
            if use_act:   # ACT idle in prologue: fused bias-add evacuation
                nc.scalar.add(dslice, ps, bqk_sb[:, jt:jt + 1])
            else:
                nc.vector.tensor_add(dslice, ps, bqk_bc_sb[:, jt, :])

        def emit_v(st):
            """V (row-major) for global s-tile st (128 rows)."""
            ps = mm_ps.tile([128, 512], F32, tag="mm")
            for kc in range(KCH):
                nc.tensor.matmul(
                    ps[:, 0:128],
                    lhsT=xt_cols(kc, st * 128, (st + 1) * 128),
                    rhs=wv_sb[:, kc, :],
                    start=(kc == 0), stop=(kc == KCH - 1),
                )
            for hl in range(HPC):
                nc.vector.tensor_add(
                    v_sb[:, st, hl * 65:hl * 65 + 64],
                    ps[:, hl * 64:(hl + 1) * 64],
                    bv_bc_sb[:, hl, :])

        # ------------- collective tiles: 3 merged AllGathers -------------
        # b0 as one [128, 2048] (g-major); b1 as two [128, 1024] halves.
        ctx_local = {
            "b0": dram.tile([128, 4 * 512], BF16, tag="ctxl_b0",
                            name="ctxl_b0"),
            "b1h0": dram.tile([128, 2 * 512], BF16, tag="ctxl_b1h0",
                              name="ctxl_b1h0"),
            "b1h1": dram.tile([128, 2 * 512], BF16, tag="ctxl_b1h1",
                              name="ctxl_b1h1"),
        }
        ctx_w = {"b0": 4 * 512, "b1h0": 2 * 512, "b1h1": 2 * 512}
        ctx_all = {
            k: dram.tile([NCORES * 128, ctx_w[k]], BF16,
                         addr_space="Shared", tag=f"ctxa_{k}",
                         name=f"ctxa_{k}")
            for k in ctx_local
        }

        def ctx_dst(b, g, hl):
            """slice of the merged local-ctx tile for (b, g, head hl)"""
            if b == 0:
                t, col = ctx_local["b0"], g * 512
            else:
                t, col = ctx_local[f"b1h{g // 2}"], (g % 2) * 512
            return t[hl * 64:(hl + 1) * 64, col:col + 512]

        def emit_ag(key):
            nc.gpsimd.collective_compute(
                "AllGather", mybir.AluOpType.bypass, replica_groups=rg,
                ins=[ctx_local[key].opt()],
                outs=[ctx_all[key].opt()],
            )

        # ------------- c_proj emitters (two-phase) -------------
        # phase 1 (emit_cg): gather DMA on the sync queue -- its AllGather
        # wait only blocks later gathers, never the PE.
        # phase 2 (emit_cpmm): the matmuls, popped >=1 q-group later so the
        # gathered data is resident when the in-order PE stream reaches them.
        cg_sets = {}

        def emit_cg(b, g):
            key = "b0" if b == 0 else f"b1h{g // 2}"
            ca, w = ctx_all[key], ctx_w[key]
            col = g * 512 if b == 0 else (g % 2) * 512
            # [1024, w] rank-major rows -> [128, 8, 512] (p, c, m) slice
            src = bass.AP(tensor=ca.tensor, offset=ca.offset + col,
                          ap=[[w, 128], [128 * w, NCORES], [1, 512]])
            cg = cg_pool.tile([128, NCORES, 512], BF16, tag="cg")
            nc.sync.dma_start(out=cg, in_=src)
            cg_sets[(b, g)] = cg

        def emit_cpmm(b, g):
            """output cols [b*S + g*512, +512), transposed [j, m]."""
            cg = cg_sets.pop((b, g))
            ps = mm_ps.tile([128, 512], F32, tag="mm")
            for c in range(NCORES):
                nc.tensor.matmul(
                    ps, lhsT=wp_sb[:, c, :], rhs=cg[:, c, :],
                    start=(c == 0), stop=(c == NCORES - 1),
                )
            o = osb.tile([128, 512], F32, tag="o")
            nc.vector.tensor_add(o, ps, bp_bc_sb)
            col = b * S + g * 512
            nc.gpsimd.dma_start(out=outT[:, col:col + 512], in_=o)

        # ------------- attention -------------
        def emit_attn(b, fill, fill_per_kt, add_after_g=None, ag_after_g=None):
            """Attention for batch b.  Per-kt pipeline: scores(kt+1) is
            emitted before AV(kt).  Pops fill-units between kt steps;
            add_after_g[g] units join the queue only after g's epilogue;
            ag_after_g[g] names a collective to launch there."""
            for g in range(G_PER_B):
                n_kt = 4 * g + 4
                cps = [ctx_ps.tile([65, 512], F32, tag="ctx", name=f"cps{_hl}")
                       for _hl in range(HPC)]
                q_sl = [qT[hl * 64:(hl + 1) * 64,
                           b * S + g * 512:b * S + (g + 1) * 512]
                        for hl in range(HPC)]
                pend_av = None
                for kt in range(n_kt):
                    qo = max(kt - 4 * g, 0) * 128  # causal trim offset
                    sp = s_ps.tile([128, 2, 512], F32, tag="s")
                    pt = pt_pool.tile([128, 2, 512], BF16, tag="pt")
                    for hl in range(HPC):
                        nc.tensor.matmul(
                            sp[:, hl, qo:512],
                            lhsT=kT[hl * 64:(hl + 1) * 64,
                                    b * S + kt * 128:b * S + (kt + 1) * 128],
                            rhs=q_sl[hl][:, qo:512],
                            start=True, stop=True,
                            tile_position=(64 * hl, 0),
                        )
                    nc.scalar.activation(pt[:, :, qo:512], sp[:, :, qo:512],
                                         EXP, scale=SCALE)
                    if kt >= 4 * g:   # diagonal block mask, both heads
                        nc.vector.tensor_mul(
                            pt[:, :, qo:qo + 128], pt[:, :, qo:qo + 128],
                            mask_sb)
                    if pend_av is not None:
                        pend_av()
                    def av(kt=kt, pt=pt, qo=qo):
                        for hl in range(HPC):
                            nc.tensor.matmul(
                                cps[hl][:, qo:512],
                                lhsT=v_sb[:, b * ST_B + kt,
                                          hl * 65:hl * 65 + 65],
                                rhs=pt[:, hl, qo:512],
                                start=(kt == 0), stop=(kt == n_kt - 1),
                                skip_group_check=True,
                            )
                    pend_av = av
                    for _ in range(fill_per_kt):
                        if fill:
                            fill.pop(0)()
                pend_av()
                # per-g normalize + ctx out: copy the sums row, broadcast it
                # across partitions with a K=1 ones matmul, fast reciprocal,
                # then scale ctx straight out of PSUM
                for hl in range(HPC):
                    row = row_pool.tile([65, 512], F32, tag="row")
                    nc.vector.tensor_copy(row[64:65, :], cps[hl][64:65, :])
                    bc_ps = mm_ps.tile([128, 512], F32, tag="mm")
                    nc.tensor.matmul(bc_ps[0:64, :], lhsT=ones_p64[64:65, :],
                                     rhs=row[64:65, :], start=True, stop=True,
                                     tile_position=(64, 0))
                    rec = rec_pool.tile([64, 512], F32, tag="rec")
                    nc.vector.reciprocal_approx_fast(rec, bc_ps[0:64, :])
                    cs = cs_pool.tile([64, 512], BF16, tag="cs")
                    nc.vector.tensor_mul(cs, cps[hl][0:64, :], rec)
                    nc.gpsimd.dma_start(out=ctx_dst(b, g, hl), in_=cs)
                if ag_after_g and g in ag_after_g:
                    emit_ag(ag_after_g[g])
                if add_after_g and g in add_after_g:
                    fill.extend(add_after_g[g])
            return fill

        # ------------- choreography -------------
        # minimal QKV prologue for attention(b0) g0 (ACT evacuation: idle)
        emit_qk(0, 0, use_act=True)
        emit_qk(0, 1, use_act=True)
        for st in range(4):
            emit_v(st)

        # rest of QKV b0 (ordered so g deps are met); all of QKV b1 moves
        # into attention(b0)'s fill queue
        fill = []
        for su in range(1, 8):
            fill.append(lambda su=su: emit_qk(su, 0))
            fill.append(lambda su=su: emit_qk(su, 1))
            for st in range(su * 4, su * 4 + 4):
                fill.append(lambda st=st: emit_v(st))

        cg_u = lambda b, g: (lambda: emit_cg(b, g))
        mm_u = lambda b, g: (lambda: emit_cpmm(b, g))
        fill = emit_attn(0, fill, 1, ag_after_g={3: "b0"})
        for f in fill:   # leftovers
            f()

        # attention b1: c_proj(b0) interleaves into g0-g3; AG(b1 h0) fires
        # after g1 so c_proj(b1 g0/g1) interleaves into g3; AG(b1 h1) after
        # g3 leaves only cg+mm of b1 g2/g3 as the tail
        fill2 = [cg_u(0, 0), cg_u(0, 1), mm_u(0, 0), cg_u(0, 2), mm_u(0, 1),
                 cg_u(0, 3), mm_u(0, 2)]
        after1 = {1: [mm_u(0, 3), cg_u(1, 0), cg_u(1, 1)],
                  2: [mm_u(1, 0)],
                  3: [mm_u(1, 1), cg_u(1, 2), cg_u(1, 3)]}
        fill2 = emit_attn(1, fill2, 1, add_after_g=after1,
                          ag_after_g={1: "b1h0", 3: "b1h1"})
        for f in fill2:
            f()
        emit_cpmm(1, 2)
        emit_cpmm(1, 3)


_CACHE = {}


def _get_compiled():
    if "nc" not in _CACHE:
        nc = bacc.Bacc("TRN2", target_bir_lowering=False, debug=False,
                       num_devices=NCORES)
        build_ir(nc)
        nc.compile()
        _CACHE["nc"] = nc
    return _CACHE["nc"]


def make_in_maps(inputs):
    x = np.asarray(inputs["hidden_states"], dtype=np.float32)   # [B,S,D]
    wa = np.asarray(inputs["c_attn_w"], dtype=np.float32)       # [D, 3D]
    ba = np.asarray(inputs["c_attn_b"], dtype=np.float32)       # [3D]
    wpr = np.asarray(inputs["c_proj_w"], dtype=np.float32)      # [D, D]
    bpr = np.asarray(inputs["c_proj_b"], dtype=np.float32)      # [D]

    bf = ml_dtypes.bfloat16
    xT = np.ascontiguousarray(x.reshape(M, D).T).astype(bf)     # [D, M]
    wq, wk, wv_full = wa[:, 0:D], wa[:, D:2 * D], wa[:, 2 * D:3 * D]
    bq, bk, bv_full = ba[0:D], ba[D:2 * D], ba[2 * D:3 * D]

    in_maps = []
    for r in range(NCORES):
        hs = slice(r * HPC * HD, (r + 1) * HPC * HD)   # this core's head dims
        bqk_r = np.concatenate([bq[hs], bk[hs]])       # [256]
        bp_r = bpr[r * 128:(r + 1) * 128]              # [128]
        bv_r = bv_full[hs]                             # [128]
        in_maps.append({
            "xt": xT,
            "wqk": np.ascontiguousarray(
                np.concatenate([wq[:, hs], wk[:, hs]], axis=1)).astype(bf),
            "wv": np.ascontiguousarray(wv_full[:, hs]).astype(bf),
            "wp": np.ascontiguousarray(wpr[:, r * 128:(r + 1) * 128]).astype(bf),
            "bqk": np.ascontiguousarray(bqk_r),
            "bqk_bc": np.ascontiguousarray(np.broadcast_to(
                bqk_r.reshape(2, 128).transpose(1, 0)[:, :, None],
                (128, 2, 512))).astype(np.float32),
            "bv_bc": np.ascontiguousarray(np.broadcast_to(
                bv_r.reshape(2, 64)[None, :, :], (128, 2, 64))).astype(bf),
            "bp_bc": np.ascontiguousarray(np.broadcast_to(
                bp_r[:, None], (128, 512))).astype(np.float32),
        })
    return in_maps


def assemble(results):
    slices = [results[r]["outT"].T.reshape(B, S, 128) for r in range(NCORES)]
    return np.ascontiguousarray(np.concatenate(slices, axis=2).astype(np.float32))


def kernel(**inputs):
    in_maps = make_in_maps(inputs)
    nc = _get_compiled()
    res = run_bass_kernel_spmd(nc, in_maps, core_ids=list(range(NCORES)))
    return assemble(res.results)


if __name__ == "__main__":
    import reference
    inp = reference.setup_inputs()
    out = kernel(**{k: np.asarray(v) for k, v in inp.items()})
    print(out.shape, out.dtype)


# revision 19
# speedup vs baseline: 1.0570x; 1.0570x over previous
"""Trainium2 Bass kernel for ClassicAttention (B=2, S=2048, D=1024, H=16).

Sharding: tensor-parallel over heads across 8 cores (2 heads/core).
  - Host pre-transposes x to x^T [D, M] and pre-casts all matmul operands
    to bf16; biases arrive as pre-broadcast SBUF tiles so no K=1 seed
    matmuls are needed (bias fused into the PSUM evacuation add).
  - QKV projection: each core computes Q^T,K^T (d-major) and V (row-major)
    for its 2 heads over all B*S rows straight from x^T in SBUF.
  - Attention: transposed-scores formulation S^T[k,q]; both heads share one
    [128,1024] score tile so each k-tile needs a single exp ACTIVATE.  The
    softmax denominator rides row 64 of the AV accumulator via a ones
    column in V.  AV matmuls are causally trimmed to [qo:512] like the
    scores (skip_group_check since the trimmed stop flag only covers the
    tail region).  Per-kt software pipeline: scores(kt+1) is emitted before
    AV(kt) so the exp(kt) runs while the PE does useful work.  Per-q-group
    normalization broadcasts the sums row with a K=1 ones matmul.
  - c_proj: 3 merged AllGathers of bf16 context (b0 whole: 512KB/rank;
    b1 in two halves: 256KB/rank) instead of 8 small ones -- the collective
    engine serializes AGs at a high fixed cost, so fewer/bigger ops keep it
    off the critical path.  Gathers land on the sync DMA queue; each core
    computes a 128-column slice of the output, transposed ([j, B*S]); the
    host transposes back.
  - Emission order software-pipelines phases: QKV(b1) matmuls interleave
    into attention(b0)'s PE stream, c_proj units into attention(b1).
  - Input DMAs spread across engine queues so QKV(b0) starts ~5us in.
All matmuls bf16 inputs with fp32 PSUM accumulation.
"""

import numpy as np
import ml_dtypes

import concourse.bass as bass
import concourse.tile as tile
import concourse.mybir as mybir
from concourse import bacc
from concourse.bass_utils import run_bass_kernel_spmd

F32 = mybir.dt.float32
BF16 = mybir.dt.bfloat16

NCORES = 8
B, S, D = 2, 2048, 1024
H, HD = 16, 64
HPC = H // NCORES          # heads per core = 2
M = B * S                  # 4096 rows
ST_B = S // 128            # 16 s-tiles per batch
KCH = D // 128             # 8 contraction chunks
G_PER_B = S // 512         # 4 q-supers per batch
SCALE = 1.0 / (HD ** 0.5)
EXP = mybir.ActivationFunctionType.Exp


def build_ir(nc):
    # ---------------- DRAM I/O ----------------
    xt = nc.dram_tensor("xt", [D, M], BF16, kind="ExternalInput").ap()
    wqk = nc.dram_tensor("wqk", [D, 256], BF16, kind="ExternalInput").ap()
    wv = nc.dram_tensor("wv", [D, 128], BF16, kind="ExternalInput").ap()
    wp = nc.dram_tensor("wp", [D, 128], BF16, kind="ExternalInput").ap()
    bqk = nc.dram_tensor("bqk", [256], F32, kind="ExternalInput").ap()
    # one tiny row [bqk(256) | bv(128) | bp(128)] bf16; the [128,*] broadcast
    # bias tiles are built on-device with K=1 matmuls (cheaper than DMAing
    # 0.8MB of pre-broadcast tiles through the startup HBM crunch)
    brows = nc.dram_tensor("brows", [1, 512], BF16, kind="ExternalInput").ap()
    outT = nc.dram_tensor("outT", [128, M], F32, kind="ExternalOutput").ap()

    # causal mask for the diagonal 128-block: mask[k, c] = 1 if c >= k,
    # duplicated for both heads ([128, 2, 128]) so one mul covers a kt tile
    mask_np = (np.arange(128)[None, :] >= np.arange(128)[:, None])
    mask2 = np.stack([mask_np, mask_np], axis=1)
    mask_const = nc.inline_tensor(mask2.astype(ml_dtypes.bfloat16),
                                  "mask_const").ap()

    rg = [list(range(NCORES))]

    with tile.TileContext(nc) as tc:
        _emit(nc, tc, xt, wqk, wv, wp, bqk, brows, outT, mask_const, rg)
    return nc


def _emit(nc, tc, xt, wqk, wv, wp, bqk, brows, outT, mask_const, rg):
    import contextlib
    es = contextlib.ExitStack()
    with es:
        singles = es.enter_context(tc.tile_pool(name="singles", bufs=1))
        dram = es.enter_context(tc.tile_pool(name="dram", bufs=1, space="DRAM"))

        # ------------- persistent SBUF -------------
        qT = singles.tile([128, M], BF16, tag="qT")
        kT = singles.tile([128, M], BF16, tag="kT")
        v_sb = singles.tile([128, B * ST_B, 130], BF16, tag="v_sb")
        mask_sb = singles.tile([128, 2, 128], BF16, tag="mask_sb")
        wqk_sb = singles.tile([128, KCH, 256], BF16, tag="wqk_sb")
        wv_sb = singles.tile([128, KCH, 128], BF16, tag="wv_sb")
        wp_sb = singles.tile([128, KCH, 128], BF16, tag="wp_sb")
        bqk_sb = singles.tile([128, 2], F32, tag="bqk_sb")
        brows_sb = singles.tile([1, 512], BF16, tag="brows_sb")
        bqk_bc_sb = singles.tile([128, 2, 512], BF16, tag="bqk_bc_sb")
        bv_bc_sb = singles.tile([128, 2, 64], BF16, tag="bv_bc_sb")
        bp_bc_sb = singles.tile([128, 512], BF16, tag="bp_bc_sb")
        ones_p64 = singles.tile([65, 64], F32, tag="ones_p64")
        ones512 = singles.tile([1, 512], BF16, tag="ones512")

        # input DMAs spread across the three DMA-capable queues (sync,
        # scalar, gpsimd), priority-ordered: the first attention group needs
        # wqk + x^T(su0) + mask + bias rows, so those lead their queues.
        nc.scalar.dma_start(out=brows_sb, in_=brows)
        nc.scalar.dma_start(out=wqk_sb,
                            in_=wqk.rearrange("(c p) j -> p c j", p=128))
        nc.scalar.dma_start(out=bqk_sb,
                            in_=bqk.rearrange("(t p) -> p t", p=128))
        nc.scalar.dma_start(out=wv_sb,
                            in_=wv.rearrange("(c p) j -> p c j", p=128))
        nc.scalar.dma_start(out=wp_sb,
                            in_=wp.rearrange("(c p) j -> p c j", p=128))
        nc.gpsimd.dma_start(out=mask_sb, in_=mask_const)
        nc.vector.memset(ones512, 1.0)
        nc.vector.memset(ones_p64, 1.0)
        # ones columns of V (cols 64 and 129); data cols are written by evac
        nc.vector.memset(v_sb[:, :, 64:65], 1.0)
        nc.vector.memset(v_sb[:, :, 129:130], 1.0)

        # PE warmup K=1 matmuls while DMAs stream, then build the broadcast
        # bias tiles on-device: [128,512] = outer(ones, bias_row) etc.
        with tc.tile_pool(name="warm_ps", bufs=1, space="PSUM") as warm_ps:
            wt = warm_ps.tile([128, 512], F32)
            for _ in range(8):
                nc.tensor.matmul(wt, lhsT=ones512[:, 0:128],
                                 rhs=ones512, start=True, stop=True)
            for jt in range(2):
                nc.tensor.matmul(wt, lhsT=brows_sb[:, jt * 128:(jt + 1) * 128],
                                 rhs=ones512, start=True, stop=True)
                nc.vector.tensor_copy(bqk_bc_sb[:, jt, :], wt)
            nc.tensor.matmul(wt[:, 0:128], lhsT=ones512[:, 0:128],
                             rhs=brows_sb[:, 256:384], start=True, stop=True)
            for hl in range(HPC):
                nc.vector.tensor_copy(bv_bc_sb[:, hl, :],
                                      wt[:, hl * 64:(hl + 1) * 64])
            nc.tensor.matmul(wt, lhsT=brows_sb[:, 384:512],
                             rhs=ones512, start=True, stop=True)
            nc.vector.tensor_copy(bp_bc_sb, wt)

        # x^T: two [128, 8, 2048] tiles (m-halves); b0 slices on the sync
        # queue, b1 slices on the gpsimd queue
        xt_r = xt.rearrange("(c p) m -> p c m", p=128)
        xt_h = {h: singles.tile([128, KCH, S], BF16, tag=f"xt_h{h}",
                                name=f"xt_h{h}") for h in range(2)}
        for su in range(8):
            h, o = su // 4, (su % 4) * 512
            eng = nc.sync if h == 0 else nc.gpsimd
            eng.dma_start(out=xt_h[h][:, :, o:o + 512],
                          in_=xt_r[:, :, h * S + o:h * S + o + 512])

        def xt_cols(c, m0, m1):
            """slice of x^T chunk c for global columns [m0, m1)"""
            h = m0 // S
            assert m1 <= (h + 1) * S
            return xt_h[h][:, c, m0 - h * S:m1 - h * S]

        # ------------- shared psum pools (8 banks total) -------------
        s_ps = es.enter_context(tc.tile_pool(name="s_ps", bufs=2, space="PSUM"))
        ctx_ps = es.enter_context(tc.tile_pool(name="ctx_ps", bufs=2, space="PSUM"))
        mm_ps = es.enter_context(tc.tile_pool(name="mm_ps", bufs=2, space="PSUM"))

        pt_pool = es.enter_context(tc.tile_pool(name="pt", bufs=4))
        row_pool = es.enter_context(tc.tile_pool(name="row", bufs=2))
        rec_pool = es.enter_context(tc.tile_pool(name="rec", bufs=2))
        cs_pool = es.enter_context(tc.tile_pool(name="cs", bufs=4))
        cg_pool = es.enter_context(tc.tile_pool(name="cg", bufs=3))
        osb = es.enter_context(tc.tile_pool(name="osb", bufs=3))

        # ------------- QKV emitters -------------
        def emit_qk(su, jt, use_act=False):
            """Q^T (jt=0) or K^T (jt=1) for row-super su (512 cols)."""
            dst = qT if jt == 0 else kT
            ps = mm_ps.tile([128, 512], F32, tag="mm")
            for kc in range(KCH):
                nc.tensor.matmul(
                    ps,
                    lhsT=wqk_sb[:, kc, jt * 128:(jt + 1) * 128],
                    rhs=xt_cols(kc, su * 512, (su + 1) * 512),
                    start=(kc == 0), stop=(kc == KCH - 1),
                )
            dslice = dst[:, su * 512:(su + 1) * 512]
            if use_act:   # ACT idle in prologue: fused bias-add evacuation
                nc.scalar.add(dslice, ps, bqk_sb[:, jt:jt + 1])
            else:
                nc.vector.tensor_add(dslice, ps, bqk_bc_sb[:, jt, :])

        def emit_v(st):
            """V (row-major) for global s-tile st (128 rows)."""
            ps = mm_ps.tile([128, 512], F32, tag="mm")
            for kc in range(KCH):
                nc.tensor.matmul(
                    ps[:, 0:128],
                    lhsT=xt_cols(kc, st * 128, (st + 1) * 128),
                    rhs=wv_sb[:, kc, :],
                    start=(kc == 0), stop=(kc == KCH - 1),
                )
            for hl in range(HPC):
                nc.vector.tensor_add(
                    v_sb[:, st, hl * 65:hl * 65 + 64],
                    ps[:, hl * 64:(hl + 1) * 64],
                    bv_bc_sb[:, hl, :])

        # ------------- collective tiles: 4 half-batch AllGathers -------------
        # one [128, 1024] tile per (batch, group-pair): 256KB/rank per AG --
        # big enough to amortize the ~8us collective floor (bus bw ~114GB/s
        # measured vs ~50GB/s for per-group 128KB AGs), small enough that
        # each AG fires well before its c_proj consumer.
        keys = [f"b{b}h{h}" for b in range(B) for h in range(2)]
        ctx_local = {k: dram.tile([128, 2 * 512], BF16, tag=f"ctxl_{k}",
                                  name=f"ctxl_{k}") for k in keys}
        ctx_all = {k: dram.tile([NCORES * 128, 2 * 512], BF16,
                                addr_space="Shared", tag=f"ctxa_{k}",
                                name=f"ctxa_{k}") for k in keys}

        def ctx_dst(b, g, hl):
            """slice of the merged local-ctx tile for (b, g, head hl)"""
            t = ctx_local[f"b{b}h{g // 2}"]
            col = (g % 2) * 512
            return t[hl * 64:(hl + 1) * 64, col:col + 512]

        def emit_ag(key):
            nc.gpsimd.collective_compute(
                "AllGather", mybir.AluOpType.bypass, replica_groups=rg,
                ins=[ctx_local[key].opt()],
                outs=[ctx_all[key].opt()],
            )

        # ------------- c_proj emitters (two-phase) -------------
        # phase 1 (emit_cg): gather DMA on the sync queue -- its AllGather
        # wait only blocks later gathers, never the PE.
        # phase 2 (emit_cpmm): the matmuls, popped >=1 q-group later so the
        # gathered data is resident when the in-order PE stream reaches them.
        cg_sets = {}

        def emit_cg(b, g):
            ca, w = ctx_all[f"b{b}h{g // 2}"], 2 * 512
            col = (g % 2) * 512
            # [1024, w] rank-major rows -> [128, 8, 512] (p, c, m) slice
            src = bass.AP(tensor=ca.tensor, offset=ca.offset + col,
                          ap=[[w, 128], [128 * w, NCORES], [1, 512]])
            cg = cg_pool.tile([128, NCORES, 512], BF16, tag="cg")
            nc.sync.dma_start(out=cg, in_=src)
            cg_sets[(b, g)] = cg

        def emit_cpmm(b, g):
            """output cols [b*S + g*512, +512), transposed [j, m]."""
            cg = cg_sets.pop((b, g))
            ps = mm_ps.tile([128, 512], F32, tag="mm")
            for c in range(NCORES):
                nc.tensor.matmul(
                    ps, lhsT=wp_sb[:, c, :], rhs=cg[:, c, :],
                    start=(c == 0), stop=(c == NCORES - 1),
                )
            o = osb.tile([128, 512], F32, tag="o")
            nc.vector.tensor_add(o, ps, bp_bc_sb)
            col = b * S + g * 512
            nc.gpsimd.dma_start(out=outT[:, col:col + 512], in_=o)

        # ------------- attention -------------
        def emit_attn(b, fill, fill_per_kt, add_after_g=None, ag_after_g=None,
                      g_order=None):
            """Attention for batch b.  Per-kt pipeline: scores(kt+1) is
            emitted before AV(kt).  Pops fill-units between kt steps;
            add_after_g[g] units join the queue only after g's epilogue;
            ag_after_g[g] names a collective to launch there."""
            for g in (g_order or range(G_PER_B)):
                n_kt = 4 * g + 4
                cps = [ctx_ps.tile([65, 512], F32, tag="ctx", name=f"cps{_hl}")
                       for _hl in range(HPC)]
                q_sl = [qT[hl * 64:(hl + 1) * 64,
                           b * S + g * 512:b * S + (g + 1) * 512]
                        for hl in range(HPC)]
                pend_av = None
                for kt in range(n_kt):
                    qo = max(kt - 4 * g, 0) * 128  # causal trim offset
                    sp = s_ps.tile([128, 2, 512], F32, tag="s")
                    pt = pt_pool.tile([128, 2, 512], BF16, tag="pt")
                    for hl in range(HPC):
                        nc.tensor.matmul(
                            sp[:, hl, qo:512],
                            lhsT=kT[hl * 64:(hl + 1) * 64,
                                    b * S + kt * 128:b * S + (kt + 1) * 128],
                            rhs=q_sl[hl][:, qo:512],
                            start=True, stop=True,
                            tile_position=(64 * hl, 0),
                        )
                    nc.scalar.activation(pt[:, :, qo:512], sp[:, :, qo:512],
                                         EXP, scale=SCALE)
                    if kt >= 4 * g:   # diagonal block mask, both heads
                        nc.vector.tensor_mul(
                            pt[:, :, qo:qo + 128], pt[:, :, qo:qo + 128],
                            mask_sb)
                    if pend_av is not None:
                        pend_av()
                    def av(kt=kt, pt=pt, qo=qo):
                        for hl in range(HPC):
                            nc.tensor.matmul(
                                cps[hl][:, qo:512],
                                lhsT=v_sb[:, b * ST_B + kt,
                                          hl * 65:hl * 65 + 65],
                                rhs=pt[:, hl, qo:512],
                                start=(kt == 0), stop=(kt == n_kt - 1),
                                skip_group_check=True,
                            )
                    pend_av = av
                    for _ in range(fill_per_kt):
                        if fill:
                            fill.pop(0)()
                pend_av()
                # per-g normalize + ctx out: copy the sums row, broadcast it
                # across partitions with a K=1 ones matmul, fast reciprocal,
                # then scale ctx straight out of PSUM
                for hl in range(HPC):
                    row = row_pool.tile([65, 512], F32, tag="row")
                    nc.vector.tensor_copy(row[64:65, :], cps[hl][64:65, :])
                    bc_ps = mm_ps.tile([128, 512], F32, tag="mm")
                    nc.tensor.matmul(bc_ps[0:64, :], lhsT=ones_p64[64:65, :],
                                     rhs=row[64:65, :], start=True, stop=True,
                                     tile_position=(64, 0))
                    rec = rec_pool.tile([64, 512], F32, tag="rec")
                    nc.vector.reciprocal_approx_fast(rec, bc_ps[0:64, :])
                    cs = cs_pool.tile([64, 512], BF16, tag="cs")
                    nc.vector.tensor_mul(cs, cps[hl][0:64, :], rec)
                    nc.gpsimd.dma_start(out=ctx_dst(b, g, hl), in_=cs)
                if ag_after_g and g in ag_after_g:
                    emit_ag(ag_after_g[g])
                if add_after_g and g in add_after_g:
                    fill.extend(add_after_g[g])
            return fill

        # ------------- choreography -------------
        # minimal QKV prologue for attention(b0) g0 (ACT evacuation: idle)
        emit_qk(0, 0, use_act=True)
        emit_qk(0, 1, use_act=True)
        for st in range(4):
            emit_v(st)

        # attention(b0) fills: first the rest of b0's QKV (in g-dependency
        # order), then b1's QKV ordered for b1's DESCENDING group schedule:
        # g3 runs first there, so Q(su7), all of K(su4-7) and V(st16-31)
        # lead; Q su6/su5/su4 (for g2/g1/g0) trail.
        fill = []
        for su in range(1, 4):
            fill.append(lambda su=su: emit_qk(su, 0))
            fill.append(lambda su=su: emit_qk(su, 1))
            for st in range(su * 4, su * 4 + 4):
                fill.append(lambda st=st: emit_v(st))
        fill.append(lambda: emit_qk(7, 0))
        for su in range(4, 8):
            fill.append(lambda su=su: emit_qk(su, 1))
            for st in range(su * 4, su * 4 + 4):
                fill.append(lambda st=st: emit_v(st))
        for su in (6, 5, 4):
            fill.append(lambda su=su: emit_qk(su, 0))

        cg_u = lambda b, g: (lambda: emit_cg(b, g))
        mm_u = lambda b, g: (lambda: emit_cpmm(b, g))
        fill = emit_attn(0, fill, 1, ag_after_g={1: "b0h0", 3: "b0h1"})
        for f in fill:   # leftovers
            f()

        # attention b1 descending (g3 -> g0): AG(b0h0) is long done, so
        # c_proj(b0 h0) interleaves into g3; AG(b0h1) (fired at b0's end)
        # completes ~kt14, so c_proj(b0 h1) pops right after g3; AG(b1h1)
        # fires after g2 and c_proj(b1 h1) runs post-attention under
        # AG(b1h0), leaving only cg+mm of b1 g1/g0 exposed as tail
        fill2 = [cg_u(0, 0), cg_u(0, 1), mm_u(0, 0), mm_u(0, 1),
                 cg_u(0, 3), cg_u(0, 2)]
        after1 = {3: [mm_u(0, 3), mm_u(0, 2)],
                  2: [cg_u(1, 3), cg_u(1, 2)]}
        fill2 = emit_attn(1, fill2, 1, add_after_g=after1,
                          ag_after_g={2: "b1h1", 0: "b1h0"},
                          g_order=[3, 2, 1, 0])
        for f in fill2:
            f()
        emit_cpmm(1, 3)
        emit_cpmm(1, 2)
        emit_cg(1, 1)
        emit_cg(1, 0)
        emit_cpmm(1, 1)
        emit_cpmm(1, 0)


_CACHE = {}


def _get_compiled():
    if "nc" not in _CACHE:
        nc = bacc.Bacc("TRN2", target_bir_lowering=False, debug=False,
                       num_devices=NCORES)
        build_ir(nc)
        nc.compile()
        _CACHE["nc"] = nc
    return _CACHE["nc"]


def make_in_maps(inputs):
    x = np.asarray(inputs["hidden_states"], dtype=np.float32)   # [B,S,D]
    wa = np.asarray(inputs["c_attn_w"], dtype=np.float32)       # [D, 3D]
    ba = np.asarray(inputs["c_attn_b"], dtype=np.float32)       # [3D]
    wpr = np.asarray(inputs["c_proj_w"], dtype=np.float32)      # [D, D]
    bpr = np.asarray(inputs["c_proj_b"], dtype=np.float32)      # [D]

    bf = ml_dtypes.bfloat16
    xT = np.ascontiguousarray(x.reshape(M, D).T).astype(bf)     # [D, M]
    wq, wk, wv_full = wa[:, 0:D], wa[:, D:2 * D], wa[:, 2 * D:3 * D]
    bq, bk, bv_full = ba[0:D], ba[D:2 * D], ba[2 * D:3 * D]

    in_maps = []
    for r in range(NCORES):
        hs = slice(r * HPC * HD, (r + 1) * HPC * HD)   # this core's head dims
        bqk_r = np.concatenate([bq[hs], bk[hs]])       # [256]
        bp_r = bpr[r * 128:(r + 1) * 128]              # [128]
        bv_r = bv_full[hs]                             # [128]
        in_maps.append({
            "xt": xT,
            "wqk": np.ascontiguousarray(
                np.concatenate([wq[:, hs], wk[:, hs]], axis=1)).astype(bf),
            "wv": np.ascontiguousarray(wv_full[:, hs]).astype(bf),
            "wp": np.ascontiguousarray(wpr[:, r * 128:(r + 1) * 128]).astype(bf),
            "bqk": np.ascontiguousarray(bqk_r),
            "brows": np.ascontiguousarray(np.concatenate(
                [bqk_r, bv_r, bp_r]).reshape(1, 512)).astype(bf),
        })
    return in_maps


def assemble(results):
    slices = [results[r]["outT"].T.reshape(B, S, 128) for r in range(NCORES)]
    return np.ascontiguousarray(np.concatenate(slices, axis=2).astype(np.float32))


def kernel(**inputs):
    in_maps = make_in_maps(inputs)
    nc = _get_compiled()
    res = run_bass_kernel_spmd(nc, in_maps, core_ids=list(range(NCORES)))
    return assemble(res.results)


if __name__ == "__main__":
    import reference
    inp = reference.setup_inputs()
    out = kernel(**{k: np.asarray(v) for k, v in inp.items()})
    print(out.shape, out.dtype)


# revision 20
# speedup vs baseline: 1.3420x; 1.2696x over previous
"""Trainium2 Bass kernel for ClassicAttention (B=2, S=2048, D=1024, H=16).

Sharding: tensor-parallel over heads across 8 cores (2 heads/core), with
NO on-device collectives: each core computes a partial c_proj output from
its own heads' context (c_proj input rows are head dims, per the TP-head
sharding), DMAs the [1024, M] f32 partial to DRAM, and the host sums the 8
partials during unsharding.  Profiling showed the collective engine costs
~30us per ctx AllGather when overlapped with compute (~120us total) plus
an exposed tail; the partial-sum form replaces all of that with ~50us of
fully-overlapped output DMA and an ~8us tail.

  - Host pre-transposes x to x^T [D, M] and pre-casts all matmul operands
    to bf16; broadcast bias tiles are built on-device with K=1 matmuls from
    a 1KB bias row (no seed matmuls in the steady state, bias fused into
    PSUM-evacuation adds).
  - QKV: each core computes Q^T,K^T (d-major) and V (row-major) for its 2
    heads over all B*S rows straight from x^T in SBUF.
  - Attention: transposed-scores formulation S^T[k,q]; both heads share one
    [128,1024] score tile so each k-tile needs a single exp ACTIVATE.  The
    softmax denominator rides row 64 of the AV accumulator via a ones
    column in V.  Scores AND the AV matmuls are causally trimmed to
    [qo:512] (skip_group_check for the partial stop flags).  Per-kt
    software pipeline: scores(kt+1) is emitted before AV(kt) so exp(kt)
    runs under PE work.  Normalization broadcasts the sums row with a K=1
    ones matmul + fast reciprocal; normalized ctx^T stays in SBUF ([128,
    512] per group, both heads stacked) and feeds c_proj directly.
  - c_proj: per (b,g): 8 single-shot matmuls (contraction = my 128 ctx
    dims) produce [128 j, 512 m] partials; evacuation alternates DVE/ACT;
    out-DMAs alternate the sync/gpsimd queues.
  - Emission order software-pipelines phases: QKV(b1) and c_proj units
    interleave into the attention PE stream as fill work.
All matmuls bf16 inputs with fp32 PSUM accumulation; partials f32.
"""

import numpy as np
import ml_dtypes

import concourse.bass as bass
import concourse.tile as tile
import concourse.mybir as mybir
from concourse import bacc
from concourse.bass_utils import run_bass_kernel_spmd

F32 = mybir.dt.float32
BF16 = mybir.dt.bfloat16

NCORES = 8
B, S, D = 2, 2048, 1024
H, HD = 16, 64
HPC = H // NCORES          # heads per core = 2
M = B * S                  # 4096 rows
ST_B = S // 128            # 16 s-tiles per batch
KCH = D // 128             # 8 contraction chunks
G_PER_B = S // 512         # 4 q-supers per batch
SCALE = 1.0 / (HD ** 0.5)
EXP = mybir.ActivationFunctionType.Exp


def build_ir(nc):
    # ---------------- DRAM I/O ----------------
    xt = nc.dram_tensor("xt", [D, M], BF16, kind="ExternalInput").ap()
    wqk = nc.dram_tensor("wqk", [D, 256], BF16, kind="ExternalInput").ap()
    wv = nc.dram_tensor("wv", [D, 128], BF16, kind="ExternalInput").ap()
    wp = nc.dram_tensor("wp", [128, D], BF16, kind="ExternalInput").ap()
    bqk = nc.dram_tensor("bqk", [256], F32, kind="ExternalInput").ap()
    # tiny row [bqk(256) | bv(128)] bf16; broadcast tiles built on-device
    brows = nc.dram_tensor("brows", [1, 384], BF16, kind="ExternalInput").ap()
    outP = nc.dram_tensor("outP", [D, M], F32, kind="ExternalOutput").ap()

    # causal mask for the diagonal 128-block: mask[k, c] = 1 if c >= k,
    # duplicated for both heads ([128, 2, 128]) so one mul covers a kt tile
    mask_np = (np.arange(128)[None, :] >= np.arange(128)[:, None])
    mask2 = np.stack([mask_np, mask_np], axis=1)
    mask_const = nc.inline_tensor(mask2.astype(ml_dtypes.bfloat16),
                                  "mask_const").ap()

    with tile.TileContext(nc) as tc:
        _emit(nc, tc, xt, wqk, wv, wp, bqk, brows, outP, mask_const)
    return nc


def _emit(nc, tc, xt, wqk, wv, wp, bqk, brows, outP, mask_const):
    import contextlib
    es = contextlib.ExitStack()
    with es:
        singles = es.enter_context(tc.tile_pool(name="singles", bufs=1))

        # ------------- persistent SBUF -------------
        qT = singles.tile([128, M], BF16, tag="qT")
        kT = singles.tile([128, M], BF16, tag="kT")
        v_sb = singles.tile([128, B * ST_B, 130], BF16, tag="v_sb")
        mask_sb = singles.tile([128, 2, 128], BF16, tag="mask_sb")
        wqk_sb = singles.tile([128, KCH, 256], BF16, tag="wqk_sb")
        wv_sb = singles.tile([128, KCH, 128], BF16, tag="wv_sb")
        wp_sb = singles.tile([128, KCH, 128], BF16, tag="wp_sb")
        bqk_sb = singles.tile([128, 2], F32, tag="bqk_sb")
        brows_sb = singles.tile([1, 384], BF16, tag="brows_sb")
        bqk_bc_sb = singles.tile([128, 2, 512], BF16, tag="bqk_bc_sb")
        bv_bc_sb = singles.tile([128, 2, 64], BF16, tag="bv_bc_sb")
        ones_p64 = singles.tile([65, 64], F32, tag="ones_p64")
        ones512 = singles.tile([1, 512], BF16, tag="ones512")

        # input DMAs spread across the three DMA-capable queues (sync,
        # scalar, gpsimd), priority-ordered: the first attention group needs
        # wqk + x^T(su0) + mask + bias rows, so those lead their queues.
        nc.scalar.dma_start(out=brows_sb, in_=brows)
        nc.scalar.dma_start(out=wqk_sb,
                            in_=wqk.rearrange("(c p) j -> p c j", p=128))
        nc.scalar.dma_start(out=bqk_sb,
                            in_=bqk.rearrange("(t p) -> p t", p=128))
        nc.scalar.dma_start(out=wv_sb,
                            in_=wv.rearrange("(c p) j -> p c j", p=128))
        nc.scalar.dma_start(out=wp_sb,
                            in_=wp.rearrange("p (c j) -> p c j", j=128))
        nc.gpsimd.dma_start(out=mask_sb, in_=mask_const)
        nc.vector.memset(ones512, 1.0)
        nc.vector.memset(ones_p64, 1.0)
        # ones columns of V (cols 64 and 129); data cols are written by evac
        nc.vector.memset(v_sb[:, :, 64:65], 1.0)
        nc.vector.memset(v_sb[:, :, 129:130], 1.0)

        # PE warmup K=1 matmuls while DMAs stream, then build the broadcast
        # bias tiles on-device: outer products of ones and the bias row
        with tc.tile_pool(name="warm_ps", bufs=1, space="PSUM") as warm_ps:
            wt = warm_ps.tile([128, 512], F32)
            for _ in range(8):
                nc.tensor.matmul(wt, lhsT=ones512[:, 0:128],
                                 rhs=ones512, start=True, stop=True)
            for jt in range(2):
                nc.tensor.matmul(wt, lhsT=brows_sb[:, jt * 128:(jt + 1) * 128],
                                 rhs=ones512, start=True, stop=True)
                nc.vector.tensor_copy(bqk_bc_sb[:, jt, :], wt)
            nc.tensor.matmul(wt[:, 0:128], lhsT=ones512[:, 0:128],
                             rhs=brows_sb[:, 256:384], start=True, stop=True)
            for hl in range(HPC):
                nc.vector.tensor_copy(bv_bc_sb[:, hl, :],
                                      wt[:, hl * 64:(hl + 1) * 64])

        # x^T: two [128, 8, 2048] tiles (m-halves); b0 slices on the sync
        # queue, b1 slices on the gpsimd queue
        xt_r = xt.rearrange("(c p) m -> p c m", p=128)
        xt_h = {h: singles.tile([128, KCH, S], BF16, tag=f"xt_h{h}",
                                name=f"xt_h{h}") for h in range(2)}
        for su in range(8):
            h, o = su // 4, (su % 4) * 512
            eng = nc.sync if h == 0 else nc.gpsimd
            eng.dma_start(out=xt_h[h][:, :, o:o + 512],
                          in_=xt_r[:, :, h * S + o:h * S + o + 512])

        def xt_cols(c, m0, m1):
            """slice of x^T chunk c for global columns [m0, m1)"""
            h = m0 // S
            assert m1 <= (h + 1) * S
            return xt_h[h][:, c, m0 - h * S:m1 - h * S]

        # ------------- shared psum pools (8 banks total) -------------
        s_ps = es.enter_context(tc.tile_pool(name="s_ps", bufs=2, space="PSUM"))
        ctx_ps = es.enter_context(tc.tile_pool(name="ctx_ps", bufs=2, space="PSUM"))
        mm_ps = es.enter_context(tc.tile_pool(name="mm_ps", bufs=2, space="PSUM"))

        pt_pool = es.enter_context(tc.tile_pool(name="pt", bufs=4))
        row_pool = es.enter_context(tc.tile_pool(name="row", bufs=2))
        rec_pool = es.enter_context(tc.tile_pool(name="rec", bufs=2))
        cs_pool = es.enter_context(tc.tile_pool(name="cs", bufs=3))
        osb = es.enter_context(tc.tile_pool(name="osb", bufs=4))

        # ------------- QKV emitters -------------
        def emit_qk(su, jt, use_act=False):
            """Q^T (jt=0) or K^T (jt=1) for row-super su (512 cols)."""
            dst = qT if jt == 0 else kT
            ps = mm_ps.tile([128, 512], F32, tag="mm")
            for kc in range(KCH):
                nc.tensor.matmul(
                    ps,
                    lhsT=wqk_sb[:, kc, jt * 128:(jt + 1) * 128],
                    rhs=xt_cols(kc, su * 512, (su + 1) * 512),
                    start=(kc == 0), stop=(kc == KCH - 1),
                )
            dslice = dst[:, su * 512:(su + 1) * 512]
            if use_act:   # ACT idle in prologue: fused bias-add evacuation
                nc.scalar.add(dslice, ps, bqk_sb[:, jt:jt + 1])
            else:
                nc.vector.tensor_add(dslice, ps, bqk_bc_sb[:, jt, :])

        def emit_v(st):
            """V (row-major) for global s-tile st (128 rows)."""
            ps = mm_ps.tile([128, 512], F32, tag="mm")
            for kc in range(KCH):
                nc.tensor.matmul(
                    ps[:, 0:128],
                    lhsT=xt_cols(kc, st * 128, (st + 1) * 128),
                    rhs=wv_sb[:, kc, :],
                    start=(kc == 0), stop=(kc == KCH - 1),
                )
            for hl in range(HPC):
                nc.vector.tensor_add(
                    v_sb[:, st, hl * 65:hl * 65 + 64],
                    ps[:, hl * 64:(hl + 1) * 64],
                    bv_bc_sb[:, hl, :])

        # ------------- c_proj emitter -------------
        cs_sets = {}

        def emit_cpmm(b, g):
            """partial out^T rows [0:1024), cols [b*S+g*512, +512), from my
            128 ctx dims; evacuation alternates DVE/ACT, out-DMA queues
            alternate sync/gpsimd."""
            cs = cs_sets.pop((b, g))
            col = b * S + g * 512
            for jc in range(KCH):
                ps = mm_ps.tile([128, 512], F32, tag="mm")
                nc.tensor.matmul(ps, lhsT=wp_sb[:, jc, :], rhs=cs,
                                 start=True, stop=True)
                o = osb.tile([128, 512], F32, tag="o")
                if jc % 2 == 0:
                    nc.vector.tensor_copy(o, ps)
                else:
                    nc.scalar.mul(o, ps, 1.0)
                eng = nc.sync if jc % 2 == 0 else nc.gpsimd
                eng.dma_start(
                    out=outP[jc * 128:(jc + 1) * 128, col:col + 512], in_=o)

        # ------------- attention -------------
        def emit_attn(b, fill, fill_per_kt, add_after_g=None):
            """Attention for batch b.  Per-kt pipeline: scores(kt+1) is
            emitted before AV(kt).  Pops fill-units between kt steps;
            add_after_g[g] units join the queue only after g's epilogue."""
            for g in range(G_PER_B):
                n_kt = 4 * g + 4
                cps = [ctx_ps.tile([65, 512], F32, tag="ctx", name=f"cps{_hl}")
                       for _hl in range(HPC)]
                q_sl = [qT[hl * 64:(hl + 1) * 64,
                           b * S + g * 512:b * S + (g + 1) * 512]
                        for hl in range(HPC)]
                pend_av = None
                for kt in range(n_kt):
                    qo = max(kt - 4 * g, 0) * 128  # causal trim offset
                    sp = s_ps.tile([128, 2, 512], F32, tag="s")
                    pt = pt_pool.tile([128, 2, 512], BF16, tag="pt")
                    for hl in range(HPC):
                        nc.tensor.matmul(
                            sp[:, hl, qo:512],
                            lhsT=kT[hl * 64:(hl + 1) * 64,
                                    b * S + kt * 128:b * S + (kt + 1) * 128],
                            rhs=q_sl[hl][:, qo:512],
                            start=True, stop=True,
                            tile_position=(64 * hl, 0),
                        )
                    nc.scalar.activation(pt[:, :, qo:512], sp[:, :, qo:512],
                                         EXP, scale=SCALE)
                    if kt >= 4 * g:   # diagonal block mask, both heads
                        nc.vector.tensor_mul(
                            pt[:, :, qo:qo + 128], pt[:, :, qo:qo + 128],
                            mask_sb)
                    if pend_av is not None:
                        pend_av()
                    def av(kt=kt, pt=pt, qo=qo):
                        for hl in range(HPC):
                            nc.tensor.matmul(
                                cps[hl][:, qo:512],
                                lhsT=v_sb[:, b * ST_B + kt,
                                          hl * 65:hl * 65 + 65],
                                rhs=pt[:, hl, qo:512],
                                start=(kt == 0), stop=(kt == n_kt - 1),
                                skip_group_check=True,
                            )
                    pend_av = av
                    for _ in range(fill_per_kt):
                        if fill:
                            fill.pop(0)()
                pend_av()
                # per-g normalize: copy the sums row, broadcast it across
                # partitions with a K=1 ones matmul, fast reciprocal, then
                # scale ctx out of PSUM into the stacked [128, 512] SBUF
                # tile (head hl on partitions hl*64..) that feeds c_proj
                cs = cs_pool.tile([128, 512], BF16, tag="cs")
                for hl in range(HPC):
                    row = row_pool.tile([65, 512], F32, tag="row")
                    nc.vector.tensor_copy(row[64:65, :], cps[hl][64:65, :])
                    bc_ps = mm_ps.tile([128, 512], F32, tag="mm")
                    nc.tensor.matmul(bc_ps[0:64, :], lhsT=ones_p64[64:65, :],
                                     rhs=row[64:65, :], start=True, stop=True,
                                     tile_position=(64, 0))
                    rec = rec_pool.tile([64, 512], F32, tag="rec")
                    nc.vector.reciprocal_approx_fast(rec, bc_ps[0:64, :])
                    nc.vector.tensor_mul(cs[hl * 64:(hl + 1) * 64, :],
                                         cps[hl][0:64, :], rec)
                cs_sets[(b, g)] = cs
                if add_after_g and g in add_after_g:
                    fill.extend(add_after_g[g])
            return fill

        # ------------- choreography -------------
        # minimal QKV prologue for attention(b0) g0 (ACT evacuation: idle)
        emit_qk(0, 0, use_act=True)
        emit_qk(0, 1, use_act=True)
        for st in range(4):
            emit_v(st)

        # attention(b0) fills: rest of b0's QKV in dependency order, then
        # b1's QKV; c_proj(b0, g) units injected right after g's epilogue
        fill = []
        for su in range(1, 8):
            fill.append(lambda su=su: emit_qk(su, 0))
            fill.append(lambda su=su: emit_qk(su, 1))
            for st in range(su * 4, su * 4 + 4):
                fill.append(lambda st=st: emit_v(st))

        mm_u = lambda b, g: (lambda: emit_cpmm(b, g))
        after0 = {g: [mm_u(0, g)] for g in range(3)}
        fill = emit_attn(0, fill, 1, add_after_g=after0)
        for f in fill:   # leftovers
            f()

        after1 = {g: [mm_u(1, g)] for g in range(3)}
        fill2 = emit_attn(1, [mm_u(0, 3)], 1, add_after_g=after1)
        for f in fill2:
            f()
        emit_cpmm(1, 3)


_CACHE = {}


def _get_compiled():
    if "nc" not in _CACHE:
        nc = bacc.Bacc("TRN2", target_bir_lowering=False, debug=False,
                       num_devices=NCORES)
        build_ir(nc)
        nc.compile()
        _CACHE["nc"] = nc
    return _CACHE["nc"]


def make_in_maps(inputs):
    x = np.asarray(inputs["hidden_states"], dtype=np.float32)   # [B,S,D]
    wa = np.asarray(inputs["c_attn_w"], dtype=np.float32)       # [D, 3D]
    ba = np.asarray(inputs["c_attn_b"], dtype=np.float32)       # [3D]
    wpr = np.asarray(inputs["c_proj_w"], dtype=np.float32)      # [D, D]

    bf = ml_dtypes.bfloat16
    xT = np.ascontiguousarray(x.reshape(M, D).T).astype(bf)     # [D, M]
    wq, wk, wv_full = wa[:, 0:D], wa[:, D:2 * D], wa[:, 2 * D:3 * D]
    bq, bk, bv_full = ba[0:D], ba[D:2 * D], ba[2 * D:3 * D]

    in_maps = []
    for r in range(NCORES):
        hs = slice(r * HPC * HD, (r + 1) * HPC * HD)   # this core's head dims
        bqk_r = np.concatenate([bq[hs], bk[hs]])       # [256]
        bv_r = bv_full[hs]                             # [128]
        in_maps.append({
            "xt": xT,
            "wqk": np.ascontiguousarray(
                np.concatenate([wq[:, hs], wk[:, hs]], axis=1)).astype(bf),
            "wv": np.ascontiguousarray(wv_full[:, hs]).astype(bf),
            "wp": np.ascontiguousarray(wpr[hs, :]).astype(bf),
            "bqk": np.ascontiguousarray(bqk_r),
            "brows": np.ascontiguousarray(np.concatenate(
                [bqk_r, bv_r]).reshape(1, 384)).astype(bf),
        })
    return in_maps


def assemble(results, c_proj_b):
    acc = results[0]["outP"].astype(np.float32).copy()
    for r in range(1, NCORES):
        acc += results[r]["outP"]
    out = acc.T.reshape(B, S, D) + c_proj_b[None, None, :]
    return np.ascontiguousarray(out.astype(np.float32))


def kernel(**inputs):
    in_maps = make_in_maps(inputs)
    nc = _get_compiled()
    res = run_bass_kernel_spmd(nc, in_maps, core_ids=list(range(NCORES)))
    return assemble(res.results,
                    np.asarray(inputs["c_proj_b"], dtype=np.float32))


if __name__ == "__main__":
    import reference
    inp = reference.setup_inputs()
    out = kernel(**{k: np.asarray(v) for k, v in inp.items()})
    print(out.shape, out.dtype)


# revision 23
# speedup vs baseline: 1.3700x; 1.0209x over previous
"""Trainium2 Bass kernel for ClassicAttention (B=2, S=2048, D=1024, H=16).

Sharding: tensor-parallel over heads across 8 cores (2 heads/core), with
NO on-device collectives: each core computes a partial c_proj output from
its own heads' context (c_proj input rows are head dims, per the TP-head
sharding), DMAs the [1024, M] f32 partial to DRAM, and the host sums the 8
partials during unsharding.  Profiling showed the collective engine costs
~30us per ctx AllGather when overlapped with compute (~120us total) plus
an exposed tail; the partial-sum form replaces all of that with ~50us of
fully-overlapped output DMA and an ~8us tail.

  - Host pre-transposes x to x^T [D, M] and pre-casts all matmul operands
    to bf16; broadcast bias tiles are built on-device with K=1 matmuls from
    a 1KB bias row (no seed matmuls in the steady state, bias fused into
    PSUM-evacuation adds).
  - QKV: each core computes Q^T,K^T (d-major) and V (row-major) for its 2
    heads over all B*S rows straight from x^T in SBUF.
  - Attention: transposed-scores formulation S^T[k,q]; both heads share one
    [128,1024] score tile so each k-tile needs a single exp ACTIVATE.  The
    softmax denominator rides row 64 of the AV accumulator via a ones
    column in V.  Scores AND the AV matmuls are causally trimmed to
    [qo:512] (skip_group_check for the partial stop flags).  Per-kt
    software pipeline: scores(kt+1) is emitted before AV(kt) so exp(kt)
    runs under PE work.  Normalization broadcasts the sums row with a K=1
    ones matmul + fast reciprocal; normalized ctx^T stays in SBUF ([128,
    512] per group, both heads stacked) and feeds c_proj directly.
  - c_proj: per (b,g): 8 single-shot matmuls (contraction = my 128 ctx
    dims) produce [128 j, 512 m] partials; evacuation alternates DVE/ACT;
    out-DMAs alternate the sync/gpsimd queues.
  - Emission order software-pipelines phases: QKV(b1) and c_proj units
    interleave into the attention PE stream as fill work.
All matmuls bf16 inputs with fp32 PSUM accumulation; partials f32.
"""

import numpy as np
import ml_dtypes

import concourse.bass as bass
import concourse.tile as tile
import concourse.mybir as mybir
from concourse import bacc
from concourse.bass_utils import run_bass_kernel_spmd

F32 = mybir.dt.float32
BF16 = mybir.dt.bfloat16

NCORES = 8
B, S, D = 2, 2048, 1024
H, HD = 16, 64
HPC = H // NCORES          # heads per core = 2
M = B * S                  # 4096 rows
ST_B = S // 128            # 16 s-tiles per batch
KCH = D // 128             # 8 contraction chunks
G_PER_B = S // 512         # 4 q-supers per batch
SCALE = 1.0 / (HD ** 0.5)
EXP = mybir.ActivationFunctionType.Exp


def build_ir(nc):
    # ---------------- DRAM I/O ----------------
    xt = nc.dram_tensor("xt", [D, M], BF16, kind="ExternalInput").ap()
    wqk = nc.dram_tensor("wqk", [D, 256], BF16, kind="ExternalInput").ap()
    wv = nc.dram_tensor("wv", [D, 128], BF16, kind="ExternalInput").ap()
    wp = nc.dram_tensor("wp", [128, D], BF16, kind="ExternalInput").ap()
    bqk = nc.dram_tensor("bqk", [256], F32, kind="ExternalInput").ap()
    # tiny row [bqk(256) | bv(128)] bf16; broadcast tiles built on-device
    brows = nc.dram_tensor("brows", [1, 384], BF16, kind="ExternalInput").ap()
    outP = nc.dram_tensor("outP", [D, M], F32, kind="ExternalOutput").ap()

    # causal mask for the diagonal 128-block: mask[k, c] = 1 if c >= k,
    # duplicated for both heads ([128, 2, 128]) so one mul covers a kt tile
    mask_np = (np.arange(128)[None, :] >= np.arange(128)[:, None])
    mask2 = np.stack([mask_np, mask_np], axis=1)
    mask_const = nc.inline_tensor(mask2.astype(ml_dtypes.bfloat16),
                                  "mask_const").ap()

    with tile.TileContext(nc) as tc:
        _emit(nc, tc, xt, wqk, wv, wp, bqk, brows, outP, mask_const)
    return nc


def _emit(nc, tc, xt, wqk, wv, wp, bqk, brows, outP, mask_const):
    import contextlib
    es = contextlib.ExitStack()
    with es:
        singles = es.enter_context(tc.tile_pool(name="singles", bufs=1))

        # ------------- persistent SBUF -------------
        qT = singles.tile([128, M], BF16, tag="qT")
        kT = singles.tile([128, M], BF16, tag="kT")
        v_sb = singles.tile([128, B * ST_B, 130], BF16, tag="v_sb")
        mask_sb = singles.tile([128, 2, 128], BF16, tag="mask_sb")
        wqk_sb = singles.tile([128, KCH, 256], BF16, tag="wqk_sb")
        wv_sb = singles.tile([128, KCH, 128], BF16, tag="wv_sb")
        wp_sb = singles.tile([128, KCH, 128], BF16, tag="wp_sb")
        bqk_sb = singles.tile([128, 2], F32, tag="bqk_sb")
        brows_sb = singles.tile([1, 384], BF16, tag="brows_sb")
        bqk_bc_sb = singles.tile([128, 2, 512], BF16, tag="bqk_bc_sb")
        bv_bc_sb = singles.tile([128, 2, 64], BF16, tag="bv_bc_sb")
        ones_p64 = singles.tile([65, 64], F32, tag="ones_p64")
        ones512 = singles.tile([1, 512], BF16, tag="ones512")

        # input DMAs spread across the three DMA-capable queues (sync,
        # scalar, gpsimd), priority-ordered: the first attention group needs
        # wqk + x^T(su0) + mask + bias rows, so those lead their queues.
        nc.scalar.dma_start(out=brows_sb, in_=brows)
        nc.scalar.dma_start(out=wqk_sb,
                            in_=wqk.rearrange("(c p) j -> p c j", p=128))
        nc.scalar.dma_start(out=bqk_sb,
                            in_=bqk.rearrange("(t p) -> p t", p=128))
        nc.scalar.dma_start(out=wv_sb,
                            in_=wv.rearrange("(c p) j -> p c j", p=128))
        nc.scalar.dma_start(out=wp_sb,
                            in_=wp.rearrange("p (c j) -> p c j", j=128))
        nc.gpsimd.dma_start(out=mask_sb, in_=mask_const)
        nc.vector.memset(ones512, 1.0)
        nc.vector.memset(ones_p64, 1.0)
        # ones columns of V (cols 64 and 129); data cols are written by evac
        nc.vector.memset(v_sb[:, :, 64:65], 1.0)
        nc.vector.memset(v_sb[:, :, 129:130], 1.0)

        # PE warmup K=1 matmuls while DMAs stream, then build the broadcast
        # bias tiles on-device: outer products of ones and the bias row
        with tc.tile_pool(name="warm_ps", bufs=1, space="PSUM") as warm_ps:
            wt = warm_ps.tile([128, 512], F32)
            for _ in range(8):
                nc.tensor.matmul(wt, lhsT=ones512[:, 0:128],
                                 rhs=ones512, start=True, stop=True)
            for jt in range(2):
                nc.tensor.matmul(wt, lhsT=brows_sb[:, jt * 128:(jt + 1) * 128],
                                 rhs=ones512, start=True, stop=True)
                nc.vector.tensor_copy(bqk_bc_sb[:, jt, :], wt)
            nc.tensor.matmul(wt[:, 0:128], lhsT=ones512[:, 0:128],
                             rhs=brows_sb[:, 256:384], start=True, stop=True)
            for hl in range(HPC):
                nc.vector.tensor_copy(bv_bc_sb[:, hl, :],
                                      wt[:, hl * 64:(hl + 1) * 64])

        # x^T: two [128, 8, 2048] tiles (m-halves); slices interleaved
        # across the sync/gpsimd queues in consumption order (su0 first)
        # so early attention groups never wait on input bandwidth
        xt_r = xt.rearrange("(c p) m -> p c m", p=128)
        xt_h = {h: singles.tile([128, KCH, S], BF16, tag=f"xt_h{h}",
                                name=f"xt_h{h}") for h in range(2)}
        for su in range(8):
            h, o = su // 4, (su % 4) * 512
            eng = nc.sync if su % 2 == 0 else nc.gpsimd
            eng.dma_start(out=xt_h[h][:, :, o:o + 512],
                          in_=xt_r[:, :, h * S + o:h * S + o + 512])

        def xt_cols(c, m0, m1):
            """slice of x^T chunk c for global columns [m0, m1)"""
            h = m0 // S
            assert m1 <= (h + 1) * S
            return xt_h[h][:, c, m0 - h * S:m1 - h * S]

        # ------------- shared psum pools (8 banks total) -------------
        s_ps = es.enter_context(tc.tile_pool(name="s_ps", bufs=2, space="PSUM"))
        ctx_ps = es.enter_context(tc.tile_pool(name="ctx_ps", bufs=2, space="PSUM"))
        mm_ps = es.enter_context(tc.tile_pool(name="mm_ps", bufs=2, space="PSUM"))

        pt_pool = es.enter_context(tc.tile_pool(name="pt", bufs=4))
        row_pool = es.enter_context(tc.tile_pool(name="row", bufs=2))
        rec_pool = es.enter_context(tc.tile_pool(name="rec", bufs=2))
        cs_pool = es.enter_context(tc.tile_pool(name="cs", bufs=3))
        osb = es.enter_context(tc.tile_pool(name="osb", bufs=4))

        # ------------- QKV emitters -------------
        def emit_qk(su, jt, use_act=False):
            """Q^T (jt=0) or K^T (jt=1) for row-super su (512 cols)."""
            dst = qT if jt == 0 else kT
            ps = mm_ps.tile([128, 512], F32, tag="mm")
            for kc in range(KCH):
                nc.tensor.matmul(
                    ps,
                    lhsT=wqk_sb[:, kc, jt * 128:(jt + 1) * 128],
                    rhs=xt_cols(kc, su * 512, (su + 1) * 512),
                    start=(kc == 0), stop=(kc == KCH - 1),
                )
            dslice = dst[:, su * 512:(su + 1) * 512]
            if use_act:   # ACT idle in prologue: fused bias-add evacuation
                nc.scalar.add(dslice, ps, bqk_sb[:, jt:jt + 1])
            else:
                nc.vector.tensor_add(dslice, ps, bqk_bc_sb[:, jt, :])

        def emit_v(st):
            """V (row-major) for global s-tile st (128 rows)."""
            ps = mm_ps.tile([128, 512], F32, tag="mm")
            for kc in range(KCH):
                nc.tensor.matmul(
                    ps[:, 0:128],
                    lhsT=xt_cols(kc, st * 128, (st + 1) * 128),
                    rhs=wv_sb[:, kc, :],
                    start=(kc == 0), stop=(kc == KCH - 1),
                )
            for hl in range(HPC):
                nc.vector.tensor_add(
                    v_sb[:, st, hl * 65:hl * 65 + 64],
                    ps[:, hl * 64:(hl + 1) * 64],
                    bv_bc_sb[:, hl, :])

        # ------------- c_proj emitter -------------
        cs_sets = {}

        def emit_cpmm(b, g, jcs, use_act=False):
            """partial out^T rows [jc*128,...), cols [b*S+g*512, +512), from
            my 128 ctx dims.  During attention the evacuations stay on DVE
            (ACT is exp-critical); the post-attention tail also uses ACT."""
            cs = cs_sets[(b, g)]
            col = b * S + g * 512
            for jc in jcs:
                ps = mm_ps.tile([128, 512], F32, tag="mm")
                nc.tensor.matmul(ps, lhsT=wp_sb[:, jc, :], rhs=cs,
                                 start=True, stop=True)
                o = osb.tile([128, 512], F32, tag="o")
                if use_act and jc % 2 == 1:
                    nc.scalar.mul(o, ps, 1.0)
                else:
                    nc.vector.tensor_copy(o, ps)
                eng = nc.sync if jc % 2 == 0 else nc.gpsimd
                eng.dma_start(
                    out=outP[jc * 128:(jc + 1) * 128, col:col + 512], in_=o)

        # ------------- attention -------------
        def emit_attn(b, fill, fill_per_kt, add_after_g=None):
            """Attention for batch b.  Per-kt pipeline: scores(kt+1) is
            emitted before AV(kt).  Pops fill-units between kt steps;
            add_after_g[g] units join the queue only after g's epilogue."""
            for g in range(G_PER_B):
                n_kt = 4 * g + 4
                cps = [ctx_ps.tile([65, 512], F32, tag="ctx", name=f"cps{_hl}")
                       for _hl in range(HPC)]
                q_sl = [qT[hl * 64:(hl + 1) * 64,
                           b * S + g * 512:b * S + (g + 1) * 512]
                        for hl in range(HPC)]
                pend_av = None
                for kt in range(n_kt):
                    qo = max(kt - 4 * g, 0) * 128  # causal trim offset
                    sp = s_ps.tile([128, 2, 512], F32, tag="s")
                    pt = pt_pool.tile([128, 2, 512], BF16, tag="pt")
                    for hl in range(HPC):
                        nc.tensor.matmul(
                            sp[:, hl, qo:512],
                            lhsT=kT[hl * 64:(hl + 1) * 64,
                                    b * S + kt * 128:b * S + (kt + 1) * 128],
                            rhs=q_sl[hl][:, qo:512],
                            start=True, stop=True,
                            tile_position=(64 * hl, 0),
                        )
                    nc.scalar.activation(pt[:, :, qo:512], sp[:, :, qo:512],
                                         EXP, scale=SCALE)
                    if kt >= 4 * g:   # diagonal block mask, both heads
                        nc.vector.tensor_mul(
                            pt[:, :, qo:qo + 128], pt[:, :, qo:qo + 128],
                            mask_sb)
                    if pend_av is not None:
                        pend_av()
                    def av(kt=kt, pt=pt, qo=qo):
                        for hl in range(HPC):
                            nc.tensor.matmul(
                                cps[hl][:, qo:512],
                                lhsT=v_sb[:, b * ST_B + kt,
                                          hl * 65:hl * 65 + 65],
                                rhs=pt[:, hl, qo:512],
                                start=(kt == 0), stop=(kt == n_kt - 1),
                                skip_group_check=True,
                            )
                    pend_av = av
                    for _ in range(fill_per_kt):
                        if fill:
                            fill.pop(0)()
                pend_av()
                # per-g normalize: copy the sums row, broadcast it across
                # partitions with a K=1 ones matmul, fast reciprocal, then
                # scale ctx out of PSUM into the stacked [128, 512] SBUF
                # tile (head hl on partitions hl*64..) that feeds c_proj
                cs = cs_pool.tile([128, 512], BF16, tag="cs")
                for hl in range(HPC):
                    row = row_pool.tile([65, 512], F32, tag="row")
                    nc.vector.tensor_copy(row[64:65, :], cps[hl][64:65, :])
                    bc_ps = mm_ps.tile([128, 512], F32, tag="mm")
                    nc.tensor.matmul(bc_ps[0:64, :], lhsT=ones_p64[64:65, :],
                                     rhs=row[64:65, :], start=True, stop=True,
                                     tile_position=(64, 0))
                    rec = rec_pool.tile([64, 512], F32, tag="rec")
                    nc.vector.reciprocal_approx_fast(rec, bc_ps[0:64, :])
                    nc.vector.tensor_mul(cs[hl * 64:(hl + 1) * 64, :],
                                         cps[hl][0:64, :], rec)
                cs_sets[(b, g)] = cs
                if add_after_g and g in add_after_g:
                    fill.extend(add_after_g[g])
            return fill

        # ------------- choreography -------------
        # minimal QKV prologue for attention(b0) g0 (ACT evacuation: idle)
        emit_qk(0, 0, use_act=True)
        emit_qk(0, 1, use_act=True)
        for st in range(4):
            emit_v(st)

        # attention(b0) fills: rest of b0's QKV in dependency order plus
        # only the b1 QKV that b1's FIRST group needs (su4); the rest of
        # b1's QKV moves into b1's own fill stream (each group g of b1
        # needs kT/v of su(4+g) one group ahead -- always satisfiable).
        # c_proj units (half-size: 4 j-chunks) inject right after each
        # group's epilogue.
        qk_u = lambda su, jt: (lambda: emit_qk(su, jt))
        v_u = lambda st: (lambda: emit_v(st))
        mm_u = lambda b, g, half: (
            lambda: emit_cpmm(b, g, range(half * 4, half * 4 + 4)))

        fill = []
        for su in range(1, 5):
            fill.append(qk_u(su, 0))
            fill.append(qk_u(su, 1))
            for st in range(su * 4, su * 4 + 4):
                fill.append(v_u(st))

        after0 = {0: [mm_u(0, 0, 0), mm_u(0, 0, 1)],
                  1: [mm_u(0, 1, 0), mm_u(0, 1, 1)],
                  2: [mm_u(0, 2, 0), mm_u(0, 2, 1)]}
        fill = emit_attn(0, fill, 1, add_after_g=after0)
        for f in fill:   # leftovers
            f()

        fill2 = [mm_u(0, 3, 0), mm_u(0, 3, 1)]
        for su in range(5, 8):
            fill2.append(qk_u(su, 0))
            fill2.append(qk_u(su, 1))
            for st in range(su * 4, su * 4 + 4):
                fill2.append(v_u(st))
        after1 = {0: [mm_u(1, 0, 0), mm_u(1, 0, 1)],
                  1: [mm_u(1, 1, 0), mm_u(1, 1, 1)],
                  2: [mm_u(1, 2, 0), mm_u(1, 2, 1)]}
        fill2 = emit_attn(1, fill2, 1, add_after_g=after1)
        for f in fill2:
            f()
        emit_cpmm(1, 3, range(KCH), use_act=True)


_CACHE = {}


def _get_compiled():
    if "nc" not in _CACHE:
        nc = bacc.Bacc("TRN2", target_bir_lowering=False, debug=False,
                       num_devices=NCORES)
        build_ir(nc)
        nc.compile()
        _CACHE["nc"] = nc
    return _CACHE["nc"]


def make_in_maps(inputs):
    x = np.asarray(inputs["hidden_states"], dtype=np.float32)   # [B,S,D]
    wa = np.asarray(inputs["c_attn_w"], dtype=np.float32)       # [D, 3D]
    ba = np.asarray(inputs["c_attn_b"], dtype=np.float32)       # [3D]
    wpr = np.asarray(inputs["c_proj_w"], dtype=np.float32)      # [D, D]

    bf = ml_dtypes.bfloat16
    xT = np.ascontiguousarray(x.reshape(M, D).T).astype(bf)     # [D, M]
    wq, wk, wv_full = wa[:, 0:D], wa[:, D:2 * D], wa[:, 2 * D:3 * D]
    bq, bk, bv_full = ba[0:D], ba[D:2 * D], ba[2 * D:3 * D]

    in_maps = []
    for r in range(NCORES):
        hs = slice(r * HPC * HD, (r + 1) * HPC * HD)   # this core's head dims
        bqk_r = np.concatenate([bq[hs], bk[hs]])       # [256]
        bv_r = bv_full[hs]                             # [128]
        in_maps.append({
            "xt": xT,
            "wqk": np.ascontiguousarray(
                np.concatenate([wq[:, hs], wk[:, hs]], axis=1)).astype(bf),
            "wv": np.ascontiguousarray(wv_full[:, hs]).astype(bf),
            "wp": np.ascontiguousarray(wpr[hs, :]).astype(bf),
            "bqk": np.ascontiguousarray(bqk_r),
            "brows": np.ascontiguousarray(np.concatenate(
                [bqk_r, bv_r]).reshape(1, 384)).astype(bf),
        })
    return in_maps


def assemble(results, c_proj_b):
    acc = results[0]["outP"].astype(np.float32).copy()
    for r in range(1, NCORES):
        acc += results[r]["outP"]
    out = acc.T.reshape(B, S, D) + c_proj_b[None, None, :]
    return np.ascontiguousarray(out.astype(np.float32))


def kernel(**inputs):
    in_maps = make_in_maps(inputs)
    nc = _get_compiled()
    res = run_bass_kernel_spmd(nc, in_maps, core_ids=list(range(NCORES)))
    return assemble(res.results,
                    np.asarray(inputs["c_proj_b"], dtype=np.float32))


if __name__ == "__main__":
    import reference
    inp = reference.setup_inputs()
    out = kernel(**{k: np.asarray(v) for k, v in inp.items()})
    print(out.shape, out.dtype)
